# revision 25
# baseline (speedup 1.0000x reference)
"""Cross-attention layer kernel for 8 Trainium2 NeuronCores.

Reference computation (fp32, D=1024, S=2048, B=4):
    q = x @ Wq.T + bq ; k = x @ Wk.T + bk ; v = x @ Wv.T + bv
    attn = softmax(q @ k.T / 32)
    vision = attn @ v                      # [B,S,D]
    text   = attn.T @ x                    # [B,S,D]

Sharding: core c handles batch b=c//2, half h=c%2 (1024 queries AND the
same 1024 rows as its key/value half).  K^T and V are computed for the
own half only and exchanged within the core pair via a DRAM AllGather
(replica groups [[0,1],[2,3],[4,5],[6,7]]).  Key order is natural, so
the host only sums the two textT partials per batch.

Precision: bf16 everywhere on the PE (runs at the same rate as f32r but
halves every DMA byte).  Softmax skips max-subtraction (scores/32 are
bounded ~3) and the 1/rowsum is folded into the two outputs.  Outputs
are stored bf16 and widened on the host.

PE schedule (the kernel is PE-streaming-bound; 1152 N=512 matmuls):
  0) ~40 warmup matmuls on a zero tile while the first DMAs land, so
     the HAM clock-gate is at 8/8 before real work starts
  1) K^T own half -> stage -> AllGather -> ktp[4] chunk tiles
  2) V   own half -> stage -> AllGather -> vp[2]
  3) Q^T own half -> qt_sb (SBUF resident)
  4) C1: scores + exp(+rowsum) -> P_sb bf16; P^T produced by DMA-xbar
     transposes (dma_start_transpose) on the sync/scalar queues -- zero
     PE cost; xs = x_q * r prepared for phase D
  5) C2: vision matmuls from ptj[] + vp[], row-scaled bf16 evict
  6) D:  textT = (x_q * r).T @ P, 8 PSUM accumulators, bf16 evict

Host-side layouts give every big DMA 8KB-contiguous per-partition
descriptors ([p][chunk][tile][512] for x^T and the weights).
"""

import sys

import numpy as np

try:
    import concourse.bass as bass
except ImportError:  # pragma: no cover - grading env should have it on path
    sys.path.insert(0, "/opt/trn_rl_repo")
    import concourse.bass as bass

import ml_dtypes
import concourse.mybir as mybir
import concourse.tile as tile
from concourse import bacc
from concourse.bass_utils import run_bass_kernel_spmd
from concourse.masks import make_identity

F32 = mybir.dt.float32
BF16 = mybir.dt.bfloat16

B = 4          # batches
S = 2048       # sequence length
D = 1024       # model dim
SH = S // 2    # queries / keys per core
P = 128        # partitions
NT = D // P    # 8 tiles along d/e
NQ = SH // P   # 8 q-tiles per core
NK = S // P    # 16 k-tiles
NC = S // 512  # 4 512-chunks along k
SCALE = 1.0 / 32.0  # 1/sqrt(D)
N512 = 512
RG = [[0, 1], [2, 3], [4, 5], [6, 7]]
NWARM = 34


def build_program():
    nc = bacc.Bacc("TRN2", target_bir_lowering=False, debug=False, num_devices=8)

    # x_own^T as [p][kchunk][dtile][512]; weights W^T as [p][ehalf][dtile][512]
    xtq_h = nc.dram_tensor("xtq", [P, 2, NT, N512], BF16, kind="ExternalInput")
    xq_h = nc.dram_tensor("xq", [NQ, P, D], BF16, kind="ExternalInput")
    wqt_h = nc.dram_tensor("wqt", [P, 2, NT, N512], BF16, kind="ExternalInput")
    wkt_h = nc.dram_tensor("wkt", [P, 2, NT, N512], BF16, kind="ExternalInput")
    wvt_h = nc.dram_tensor("wvt", [P, 2, NT, N512], BF16, kind="ExternalInput")
    bq_h = nc.dram_tensor("bq", [D], F32, kind="ExternalInput")
    bk_h = nc.dram_tensor("bk", [D], F32, kind="ExternalInput")
    bvb_h = nc.dram_tensor("bvb", [P, D], BF16, kind="ExternalInput")

    vision_h = nc.dram_tensor("vision", [SH, D], BF16, kind="ExternalOutput")
    textT_h = nc.dram_tensor("textT", [D, S], BF16, kind="ExternalOutput")

    xtq_r = xtq_h.ap()
    xq_r = xq_h.ap()
    wq_r = wqt_h.ap()
    wk_r = wkt_h.ap()
    wv_r = wvt_h.ap()
    bq_r = bq_h.ap().rearrange("(t p) -> p t", p=P)          # [128,8]
    bk_r = bk_h.ap().rearrange("(t p) -> p t", p=P)
    bvb_r = bvb_h.ap()                                       # host-broadcast

    with tile.TileContext(nc) as tc:
        with (
            tc.tile_pool(name="singles", bufs=1) as singles,
            tc.tile_pool(name="dram", bufs=1, space="DRAM") as dram_pool,
        ):
            cc_in_k0 = dram_pool.tile([P, NT, N512], BF16)
            cc_in_k1 = dram_pool.tile([P, NT, N512], BF16)
            cc_out_k0 = dram_pool.tile([2, P, NT, N512], BF16)
            cc_out_k1 = dram_pool.tile([2, P, NT, N512], BF16)
            cc_in_v = dram_pool.tile([P, NT, D], BF16)
            cc_out_v = dram_pool.tile([2, P, NT, D], BF16)

            warm = singles.tile([P, N512], BF16)
            nc.gpsimd.memset(warm, 0.0)
            ident_f = singles.tile([P, P], F32)
            make_identity(nc, ident_f)
            ident = singles.tile([P, P], BF16)
            nc.vector.tensor_copy(ident, ident_f)

            ktp = [singles.tile([P, NT, N512], BF16, name="ktp%d" % kc)
                   for kc in range(NC)]
            vp = [singles.tile([P, NT, D], BF16, name="vp%d" % r)
                  for r in range(2)]
            qt_sb = singles.tile([P, NT, SH], BF16)    # Q^T own [e, q] 16KB
            xs = singles.tile([P, NQ, D], BF16)        # (x_q * r)  16KB
            r_all = singles.tile([P, NQ], F32)

            bq_sb = singles.tile([P, NT], F32)
            bk_sb = singles.tile([P, NT], F32)
            bvb = singles.tile([P, D], BF16)

            with (
                tc.tile_pool(name="xpool", bufs=1) as xpool,
                tc.tile_pool(name="wpool", bufs=4) as wpool,
                tc.tile_pool(name="stage", bufs=3) as stage,
                tc.tile_pool(name="proj_ps", bufs=4, space="PSUM") as proj_ps,
                tc.tile_pool(name="warm_ps", bufs=2, space="PSUM") as warm_ps,
            ):
                xtq = xpool.tile([P, 2, NT, N512], BF16, name="xtq")
                # Queue plan for the head (first MM gates on xtq c0 + wk h0).
                # wv/bvb/wq are anchored behind the kt staging copies on
                # gpsimd so they cannot steal HBM bandwidth from the gating
                # loads; the anchors release just in time for the V/Q phases.
                #   sync:   xtq c0, xtq c1, bk, bq, ktp unpacks
                #   scalar: wk h0, wk h1, then per-j xq loads in C1
                #   gpsimd: memset, [kt-stage anchor] wv/bvb/wq, AGs, vp
                nc.sync.dma_start(out=xtq[:, 0], in_=xtq_r[:, 0])

                def w_half(src_r, h, eng=None):
                    eng = eng or nc.scalar
                    wt = wpool.tile([P, NT, N512], BF16, tag="wh", name="wt")
                    eng.dma_start(out=wt, in_=src_r[:, h])
                    return wt

                wt_k0 = w_half(wk_r, 0)
                nc.sync.dma_start(out=xtq[:, 1], in_=xtq_r[:, 1])
                wt_k1 = w_half(wk_r, 1)
                nc.sync.dma_start(out=bk_sb, in_=bk_r)
                nc.sync.dma_start(out=bq_sb, in_=bq_r)

                # PE warmup while DMAs land (HAM un-throttles after ~3.4us;
                # N=512 x ~34 spans the ~13us DMA head even when cold)
                for _ in range(NWARM):
                    wps = warm_ps.tile([P, N512], F32, tag="w")
                    nc.tensor.matmul(wps, warm[:, 0:P], warm,
                                     start=True, stop=True)

                # ---- phase 1: K^T own half + AllGather ------------------
                # n-major so kt_stage_n[0] completes at 50% of the K-proj,
                # releasing the wv/bvb anchor and the first AllGather early
                kt_stage_n = [stage.tile([P, NT, N512], BF16, tag="st",
                                         name="kt_stage%d" % _n)
                              for _n in range(2)]
                for n in range(2):
                    for h2 in range(2):
                        wt = wt_k0 if h2 == 0 else wt_k1
                        for tl in range(4):
                            t = h2 * 4 + tl
                            ps = proj_ps.tile([P, N512], F32, tag="acc")
                            for td in range(NT):
                                nc.tensor.matmul(
                                    ps,
                                    wt[:, td, tl * P:(tl + 1) * P],
                                    xtq[:, n, td, :],
                                    start=(td == 0), stop=(td == NT - 1))
                            nc.scalar.activation(
                                kt_stage_n[n][:, t, :], ps,
                                mybir.ActivationFunctionType.Identity,
                                bias=bk_sb[:, t:t + 1], scale=1.0)

                nc.gpsimd.dma_start(out=cc_in_k0[:], in_=kt_stage_n[0])
                wv_halves = [w_half(wv_r, 0, eng=nc.gpsimd),
                             w_half(wv_r, 1, eng=nc.gpsimd)]
                nc.gpsimd.dma_start(out=bvb, in_=bvb_r)
                nc.gpsimd.collective_compute(
                    "AllGather", mybir.AluOpType.bypass,
                    replica_groups=RG,
                    ins=[cc_in_k0.opt()], outs=[cc_out_k0.opt()],
                )
                nc.gpsimd.dma_start(out=cc_in_k1[:], in_=kt_stage_n[1])
                wq_halves = [w_half(wq_r, 0, eng=nc.gpsimd),
                             w_half(wq_r, 1, eng=nc.gpsimd)]
                nc.gpsimd.collective_compute(
                    "AllGather", mybir.AluOpType.bypass,
                    replica_groups=RG,
                    ins=[cc_in_k1.opt()], outs=[cc_out_k1.opt()],
                )
                for n, co in ((0, cc_out_k0), (1, cc_out_k1)):
                    for r in range(2):
                        nc.sync.dma_start(out=ktp[r * 2 + n], in_=co[r])

                # ---- phase 2: V own half + AllGather --------------------
                v_stage_h = []
                for h2 in range(2):
                    wt = wv_halves[h2]
                    vst = stage.tile([P, NT, N512], BF16, tag="st",
                                     name="v_stage%d" % h2)
                    v_stage_h.append(vst)
                    for kk in range(NT):
                        ps = proj_ps.tile([P, N512], F32, tag="acc")
                        for td in range(NT):
                            nc.tensor.matmul(
                                ps,
                                xtq[:, kk // 4, td,
                                    (kk % 4) * P:((kk % 4) + 1) * P],
                                wt[:, td, :],
                                start=(td == 0), stop=(td == NT - 1))
                        nc.vector.tensor_add(
                            vst[:, kk, :], ps,
                            bvb[:, h2 * N512:(h2 + 1) * N512])
                # ---- phase 3: Q^T own half (resident) -------------------
                for h2 in range(2):
                    wt = wq_halves[h2]
                    for n in range(2):
                        for tl in range(4):
                            t = h2 * 4 + tl
                            ps = proj_ps.tile([P, N512], F32, tag="acc")
                            for td in range(NT):
                                nc.tensor.matmul(
                                    ps,
                                    wt[:, td, tl * P:(tl + 1) * P],
                                    xtq[:, n, td, :],
                                    start=(td == 0), stop=(td == NT - 1))
                            nc.scalar.activation(
                                qt_sb[:, t, n * N512:(n + 1) * N512], ps,
                                mybir.ActivationFunctionType.Identity,
                                bias=bq_sb[:, t:t + 1], scale=1.0)

                for h2 in range(2):
                    nc.gpsimd.dma_start(
                        out=cc_in_v[:, :, h2 * N512:(h2 + 1) * N512],
                        in_=v_stage_h[h2])
                nc.gpsimd.collective_compute(
                    "AllGather", mybir.AluOpType.bypass,
                    replica_groups=RG,
                    ins=[cc_in_v.opt()], outs=[cc_out_v.opt()],
                )
                # vp unpacks ride the idle gpsimd queue so they can't
                # head-of-line-block the xbar transposes on sync/scalar
                for r in range(2):
                    nc.gpsimd.dma_start(out=vp[r], in_=cc_out_v[r])

            # ---- C1: scores + exp(+rowsum); P_sb resident; xs prep ------
            with (
                tc.tile_pool(name="late", bufs=1) as late,
                tc.tile_pool(name="ptj_pool", bufs=NQ) as ptj_pool,
            ):
                P_sb = late.tile([P, NQ, S], BF16, name="P_sb")
                ptjs = [None] * NQ
                with (
                    tc.tile_pool(name="c1_l", bufs=NQ + 2) as c1_l,
                    tc.tile_pool(name="xq_in", bufs=2) as xq_in,
                    tc.tile_pool(name="c1_ps", bufs=4, space="PSUM") as c1_ps,
                    tc.tile_pool(name="tr_ps", bufs=4, space="PSUM") as tr_ps,
                ):
                    def xpose(j):
                        # P^T tile-by-tile on the PE (transpose-mode matmul);
                        # ~165ns each effective, pipelines with score chains
                        ptj = ptj_pool.tile([P, NK, P], BF16, tag="ptj",
                                            name="ptj%d" % j)
                        for i in range(NK):
                            ps = tr_ps.tile([P, P], BF16, tag="tr")
                            nc.tensor.transpose(
                                ps, P_sb[:, j, i * P:(i + 1) * P], ident)
                            nc.vector.tensor_copy(out=ptj[:, i, :], in_=ps)
                        ptjs[j] = ptj

                    def score_chain(j, kc, l4):
                        ps = c1_ps.tile([P, N512], F32, tag="s")
                        for t in range(NT):
                            nc.tensor.matmul(
                                ps,
                                qt_sb[:, t, j * P:(j + 1) * P],
                                ktp[kc][:, t, :],
                                start=(t == 0), stop=(t == NT - 1))
                        nc.scalar.activation(
                            P_sb[:, j, kc * N512:(kc + 1) * N512], ps,
                            mybir.ActivationFunctionType.Exp,
                            bias=0.0, scale=SCALE,
                            accum_out=l4[:, kc:kc + 1])

                    # pass 1+2 only need AG-k0; the per-j pass finishes each
                    # P_sb row so its xbar transpose can issue ~3.5us apart,
                    # hiding the ~13us-per-slab xbar latency under C1+C2
                    l4s = []
                    for j in range(NQ):
                        l4 = c1_l.tile([P, NC], F32, tag="l4", name="l4_%d" % j)
                        l4s.append(l4)
                        score_chain(j, 0, l4)
                    for j in range(NQ):
                        score_chain(j, 2, l4s[j])
                    xins = []
                    for j in range(2):
                        xin = xq_in.tile([P, D], BF16, tag="xin", name="xin")
                        nc.scalar.dma_start(out=xin, in_=xq_r[j])
                        xins.append(xin)
                    for j in range(NQ):
                        xin = xins[j] if j < 2 else None
                        if xin is None:
                            xin = xq_in.tile([P, D], BF16, tag="xin",
                                             name="xin")
                            nc.scalar.dma_start(out=xin, in_=xq_r[j])
                        l4 = l4s[j]
                        score_chain(j, 1, l4)
                        score_chain(j, 3, l4)
                        if j > 0:
                            xpose(j - 1)
                        lsum = c1_l.tile([P, 1], F32, tag="lsum")
                        nc.vector.reduce_sum(out=lsum, in_=l4,
                                             axis=mybir.AxisListType.X)
                        nc.vector.reciprocal(out=r_all[:, j:j + 1], in_=lsum)
                        nc.vector.tensor_scalar_mul(
                            xs[:, j, :], xin, r_all[:, j:j + 1])
                    xpose(NQ - 1)

                # ---- C2: vision ---------------------------------------------
                with (
                    tc.tile_pool(name="c2_ev", bufs=4) as c2_ev,
                    tc.tile_pool(name="c2_vp", bufs=4, space="PSUM") as c2_vp,
                ):
                    def vision(j, ptj):
                        for h2 in range(2):
                            ps = c2_vp.tile([P, N512], F32, tag="vp")
                            for i in range(NK):
                                nc.tensor.matmul(
                                    ps,
                                    ptj[:, i, :],
                                    vp[i // NT][:, i % NT,
                                                h2 * N512:(h2 + 1) * N512],
                                    start=(i == 0), stop=(i == NK - 1))
                            ev = c2_ev.tile([P, N512], BF16, tag="ev")
                            nc.vector.tensor_scalar_mul(
                                ev, ps, r_all[:, j:j + 1])
                            nc.gpsimd.dma_start(
                                out=vision_h.ap()[j * P:(j + 1) * P,
                                                  h2 * N512:(h2 + 1) * N512],
                                in_=ev)

                    for j in range(NQ):
                        vision(j, ptjs[j])

                # ---- D: textT = (x_q * r).T @ P -----------------------------
                with (
                    tc.tile_pool(name="d_ev", bufs=4) as d_ev,
                    tc.tile_pool(name="d_ps", bufs=8, space="PSUM") as d_ps,
                ):
                    for kc in range(NC):
                        for dc in range(NT):
                            ps = d_ps.tile([P, N512], F32, tag="tp")
                            for j in range(NQ):
                                nc.tensor.matmul(
                                    ps,
                                    xs[:, j, dc * P:(dc + 1) * P],
                                    P_sb[:, j, kc * N512:(kc + 1) * N512],
                                    start=(j == 0), stop=(j == NQ - 1))
                            ev = d_ev.tile([P, N512], BF16, tag="ev")
                            nc.vector.tensor_copy(out=ev, in_=ps)
                            eng = nc.sync if dc % 2 == 0 else nc.scalar
                            eng.dma_start(
                                out=textT_h.ap()[dc * P:(dc + 1) * P,
                                                 kc * N512:(kc + 1) * N512],
                                in_=ev)

    nc.compile()
    return nc


_NC_CACHE = []


def _get_program():
    if not _NC_CACHE:
        _NC_CACHE.append(build_program())
    return _NC_CACHE[0]


def _wlay(W):
    """Torch-Linear weight [e,d] -> W^T [d,e] as [p][ehalf][dtile][512]."""
    WT = np.ascontiguousarray(np.asarray(W, dtype=np.float32).T)
    return np.ascontiguousarray(
        WT.reshape(NT, P, 2, N512).transpose(1, 2, 0, 3)).astype(
            ml_dtypes.bfloat16)


def kernel(inputs, Wq, bq, Wk, bk, Wv, bv, _run_opts=None):
    x = np.asarray(inputs, dtype=np.float32)
    WqL, WkL, WvL = _wlay(Wq), _wlay(Wk), _wlay(Wv)
    bq = np.ascontiguousarray(np.asarray(bq, dtype=np.float32))
    bk = np.ascontiguousarray(np.asarray(bk, dtype=np.float32))
    bvb = np.ascontiguousarray(np.broadcast_to(
        np.asarray(bv, dtype=np.float32), (P, D))).astype(ml_dtypes.bfloat16)

    nc = _get_program()

    in_maps = []
    for c in range(8):
        b, h = divmod(c, 2)
        xo = x[b, h * SH:(h + 1) * SH]                       # [1024, 1024]
        xt = np.ascontiguousarray(xo.T)                      # [d, k]
        xtq = np.ascontiguousarray(
            xt.reshape(NT, P, 2, N512).transpose(1, 2, 0, 3)).astype(
                ml_dtypes.bfloat16)
        xqj = np.ascontiguousarray(xo.reshape(NQ, P, D)).astype(
            ml_dtypes.bfloat16)
        in_maps.append({
            "xtq": xtq, "xq": xqj,
            "wqt": WqL, "wkt": WkL, "wvt": WvL,
            "bq": bq, "bk": bk, "bvb": bvb,
        })

    run_opts = dict(_run_opts or {})
    res = run_bass_kernel_spmd(nc, in_maps, core_ids=list(range(8)), **run_opts)
    results = res.results

    vision = np.empty((B, S, D), np.float32)
    text = np.zeros((B, S, D), np.float32)
    for c in range(8):
        b, h = divmod(c, 2)
        vision[b, h * SH:(h + 1) * SH] = np.asarray(
            results[c]["vision"], dtype=np.float32)
        text[b] += np.asarray(results[c]["textT"], dtype=np.float32).T
    if _run_opts is not None:
        return (vision, text), res
    return (vision, text)
